# revision 5
# baseline (speedup 1.0000x reference)
"""Trainium2 Bass kernel for windowed (sparse) attention with memory KV.

Sequence-sharded across 8 NeuronCores: core c computes output tokens
[c*512, (c+1)*512) for both batches and all heads, with a 1-window (128
token) k/v halo. The full attn_bias is never shipped: only the block-
diagonal and sub-diagonal 128x128 blocks each core needs (pre-transposed,
mask folded in as -inf rows).

Device dataflow (per core, per batch):
  x -> PE-transpose -> xT [768, 640]
  qT = (Wq*s).T @ xT + bq*s       [1024, 512]   (feature-major)
  kT = Wk.T @ xT                  [1024, 640]
  v  = xT.T @ Wv                  [640, 1024]   (token-major, +ones col/head)
  per head pair (row-packed K=64 matmuls):
    simT chunk = kT_chunk.T @ qT  [128 keys, <=256 q]
    exp = Exp(simT + biasT)       (bias add on DVE, Exp on ACT)
    out/sumexp fused: psum[128q, 65] = exp_mem.T@mv_ext + exp_prev.T@vprev_ext
                                       + exp_cur.T@vcur_ext
    out = psum[:, :64] * recip(psum[:, 64])     (per-partition scalar)
  out_all [128q, 1024] -> PE-transpose -> y = outT.T @ Wo -> DMA out
"""

import numpy as np

B, N, DIM = 2, 4096, 768
H, DH = 16, 64
W = 128
DI = H * DH                 # 1024
NEG = -3.4028235e38
NCORES = 8
TOK = N // NCORES           # 512
NWIN = TOK // W             # 4
KTOK = TOK + W              # 640
NKC = KTOK // W             # 5
KC6 = DIM // 128            # 6 contraction chunks over DIM
DC8 = DI // 128             # 8 chunks over DI

# matmul dtype for the big projections / sim ("float32" or "float32r")
MM_DT_NAME = "float32"


def build_bass():
    import concourse.mybir as mybir
    import concourse.tile as tile
    from concourse import bacc
    from concourse.masks import make_identity
    from contextlib import ExitStack

    f32 = mybir.dt.float32
    mm_dt = getattr(mybir.dt, MM_DT_NAME)
    Exp = mybir.ActivationFunctionType.Exp
    Identity = mybir.ActivationFunctionType.Identity

    nc = bacc.Bacc("TRN2")

    xkv_d = nc.dram_tensor("xkv", [B * KTOK, DIM], f32, kind="ExternalInput")
    biasc_d = nc.dram_tensor("biasc", [B * NKC * W, 2 * W], f32, kind="ExternalInput")
    wq_d = nc.dram_tensor("wq", [DIM, DI], f32, kind="ExternalInput")
    bqs_d = nc.dram_tensor("bqs", [DC8, 128], f32, kind="ExternalInput")
    wkv_d = nc.dram_tensor("wkv", [DIM, 2 * DI], f32, kind="ExternalInput")
    wo_d = nc.dram_tensor("wo", [DI, DIM], f32, kind="ExternalInput")
    memk_d = nc.dram_tensor("memk", [128, 32], f32, kind="ExternalInput")
    memv_d = nc.dram_tensor("memv", [4, 16 * 65], f32, kind="ExternalInput")
    y_d = nc.dram_tensor("y", [B * TOK, DIM], f32, kind="ExternalOutput")

    def mm(t):
        return t.bitcast(mm_dt) if MM_DT_NAME != "float32" else t

    with ExitStack() as ctx:
        tc = ctx.enter_context(tile.TileContext(nc))
        # SBUF pools
        const_p = ctx.enter_context(tc.tile_pool(name="const", bufs=1))
        w_p = ctx.enter_context(tc.tile_pool(name="w", bufs=KC6))
        wo_p = ctx.enter_context(tc.tile_pool(name="wo", bufs=DC8))
        xs_p = ctx.enter_context(tc.tile_pool(name="xs", bufs=2))
        xt_p = ctx.enter_context(tc.tile_pool(name="xt", bufs=KC6))
        kt_p = ctx.enter_context(tc.tile_pool(name="kt", bufs=DC8))
        qt_p = ctx.enter_context(tc.tile_pool(name="qt", bufs=DC8))
        v_p = ctx.enter_context(tc.tile_pool(name="v", bufs=NKC))
        exp_p = ctx.enter_context(tc.tile_pool(name="exp", bufs=6))
        em_p = ctx.enter_context(tc.tile_pool(name="em", bufs=4))
        oa_p = ctx.enter_context(tc.tile_pool(name="oa", bufs=NWIN))
        ot_p = ctx.enter_context(tc.tile_pool(name="ot", bufs=DC8))
        y_p = ctx.enter_context(tc.tile_pool(name="y", bufs=2))
        rc_p = ctx.enter_context(tc.tile_pool(name="rc", bufs=4))
        # PSUM pools
        pp = ctx.enter_context(tc.tile_pool(name="pp", bufs=2, space="PSUM"))
        sim_p = ctx.enter_context(tc.tile_pool(name="simp", bufs=2, space="PSUM"))
        av_p = ctx.enter_context(tc.tile_pool(name="avp", bufs=2, space="PSUM"))
        tr_p = ctx.enter_context(tc.tile_pool(name="trp", bufs=2, space="PSUM"))

        ident = const_p.tile([128, 128], f32)
        make_identity(nc, ident)

        bias_sb = const_p.tile([W, B * NKC * 2 * W], f32)
        for b in range(B):
            for kc in range(NKC):
                col = (b * NKC + kc) * 2 * W
                nc.sync.dma_start(
                    bias_sb[:, col:col + 2 * W],
                    biasc_d[(b * NKC + kc) * W:(b * NKC + kc + 1) * W, :])
        memk_sb = const_p.tile([128, 32], f32)
        nc.sync.dma_start(memk_sb, memk_d[:, :])
        memv_sb = const_p.tile([4, 16 * 65], f32)
        nc.sync.dma_start(memv_sb, memv_d[:, :])
        bqs_sb = const_p.tile([128, DC8], f32)
        nc.sync.dma_start(bqs_sb, bqs_d.rearrange("c p -> p c"))
        wo_sb = [wo_p.tile([128, DIM], f32, tag="wo", name=f"wo{_}") for _ in range(DC8)]
        for d in range(DC8):
            nc.sync.dma_start(wo_sb[d], wo_d[d * 128:(d + 1) * 128, :])

        for b in range(B):
            # ---- phase 1: x transpose ----
            xT = [xt_p.tile([128, KTOK], f32, tag="xt", name=f"xt{_}") for _ in range(KC6)]
            for tt in range(NKC):
                xs = xs_p.tile([128, DIM], f32, tag="xs", name="xs")
                r0 = b * KTOK + tt * 128
                nc.sync.dma_start(xs, xkv_d[r0:r0 + 128, :])
                for d in range(KC6):
                    ps = tr_p.tile([128, 128], f32, tag="tr", name="tr")
                    nc.tensor.transpose(ps, xs[:, d * 128:(d + 1) * 128], ident)
                    nc.vector.tensor_copy(xT[d][:, tt * 128:(tt + 1) * 128], ps)

            # ---- phase 2a: kT = Wk.T @ xT ----
            wk = [w_p.tile([128, 2 * DI], f32, tag="w", name=f"w{_}") for _ in range(KC6)]
            for d in range(KC6):
                nc.sync.dma_start(wk[d][:, :DI], wkv_d[d * 128:(d + 1) * 128, :DI])
            kT = [kt_p.tile([128, KTOK], f32, tag="kt", name=f"kt{_}") for _ in range(DC8)]
            for d8 in range(DC8):
                for nt in range(2):
                    ps = pp.tile([128, 512], f32, tag="pp", name="pp")[:, :320]
                    for k6 in range(KC6):
                        nc.tensor.matmul(
                            ps, mm(wk[k6][:, d8 * 128:(d8 + 1) * 128]),
                            mm(xT[k6][:, nt * 320:(nt + 1) * 320]),
                            start=(k6 == 0), stop=(k6 == KC6 - 1))
                    nc.vector.tensor_copy(kT[d8][:, nt * 320:(nt + 1) * 320], ps)

            # ---- phase 2b: v = xT.T @ Wv (token-major, 65-strided + ones) ----
            wv = [w_p.tile([128, 2 * DI], f32, tag="w", name=f"w{_}") for _ in range(KC6)]
            for d in range(KC6):
                nc.sync.dma_start(wv[d][:, :DI], wkv_d[d * 128:(d + 1) * 128, DI:])
            v_ext = [v_p.tile([128, 16 * 65], f32, tag="v", name=f"v{_}") for _ in range(NKC)]
            for tt in range(NKC):
                v3 = v_ext[tt].rearrange("p (h c) -> p h c", c=65)
                nc.vector.memset(v3[:, :, 64:65], 1.0)
                for half in range(2):
                    ps = pp.tile([128, 512], f32, tag="pp", name="pp")
                    for k6 in range(KC6):
                        nc.tensor.matmul(
                            ps, mm(xT[k6][:, tt * 128:(tt + 1) * 128]),
                            mm(wv[k6][:, half * 512:(half + 1) * 512]),
                            start=(k6 == 0), stop=(k6 == KC6 - 1))
                    nc.vector.tensor_copy(
                        v3[:, half * 8:(half + 1) * 8, 0:64],
                        ps.rearrange("p (h c) -> p h c", c=64))

            # ---- phase 2c: qT = (Wq*s).T @ xT + bq*s ----
            wqs = [w_p.tile([128, 2 * DI], f32, tag="w", name=f"w{_}") for _ in range(KC6)]
            for d in range(KC6):
                nc.sync.dma_start(wqs[d][:, :DI], wq_d[d * 128:(d + 1) * 128, :])
            qT = [qt_p.tile([128, TOK], f32, tag="qt", name=f"qt{_}") for _ in range(DC8)]
            for d8 in range(DC8):
                ps = pp.tile([128, 512], f32, tag="pp", name="pp")
                for k6 in range(KC6):
                    nc.tensor.matmul(
                        ps, mm(wqs[k6][:, d8 * 128:(d8 + 1) * 128]),
                        mm(xT[k6][:, W:W + TOK]),
                        start=(k6 == 0), stop=(k6 == KC6 - 1))
                nc.scalar.activation(qT[d8], ps, Identity,
                                     bias=bqs_sb[:, d8:d8 + 1])

            # ---- phase 3: attention ----
            out_all = [oa_p.tile([128, DI], f32, tag="oa", name=f"oa{_}") for _ in range(NWIN)]
            for hp in range(DC8):
                # memory-key sim + exp: [4, 512] per head
                emem = []
                for h01 in range(2):
                    rows = slice(64 * h01, 64 * h01 + 64)
                    psm = pp.tile([128, 512], f32, tag="pp", name="pp")[:4]
                    nc.tensor.matmul(
                        psm, mm(memk_sb[rows, hp * 4:(hp + 1) * 4]),
                        mm(qT[hp][rows, :]), start=True, stop=True)
                    et = em_p.tile([4, 512], f32, tag="em", name="em")
                    nc.scalar.activation(et, psm, Exp)
                    emem.append(et)
                exp_tiles = {}
                for kc in range(NKC):
                    qlo = max(0, (kc - 1) * W)
                    qhi = min(TOK, (kc + 1) * W)
                    qw = qhi - qlo
                    off = qlo - (kc - 1) * W
                    for h01 in range(2):
                        rows = slice(64 * h01, 64 * h01 + 64)
                        ps = sim_p.tile([128, 256], f32, tag="sim", name="sim")[:, :qw]
                        nc.tensor.matmul(
                            ps, mm(kT[hp][rows, kc * W:(kc + 1) * W]),
                            mm(qT[hp][rows, qlo:qhi]), start=True, stop=True)
                        et = exp_p.tile([128, 256], f32, tag="exp", name="exp")[:, :qw]
                        bcol = (b * NKC + kc) * 2 * W + off
                        nc.vector.tensor_add(et, ps, bias_sb[:, bcol:bcol + qw])
                        nc.scalar.activation(et, et, Exp)
                        exp_tiles[(h01, kc)] = et
                    if kc >= 1:
                        w = kc - 1
                        for h01 in range(2):
                            hg = 2 * hp + h01
                            pcol = 0 if w == 0 else W
                            prev_e = exp_tiles[(h01, w)]
                            cur_e = exp_tiles[(h01, kc)]
                            psv = av_p.tile([128, 65], f32, tag="av", name="av")
                            nc.tensor.matmul(
                                psv, mm(emem[h01][:, w * W:(w + 1) * W]),
                                mm(memv_sb[:, hg * 65:(hg + 1) * 65]),
                                start=True, stop=False)
                            nc.tensor.matmul(
                                psv, mm(prev_e[:, pcol:pcol + W]),
                                mm(v_ext[w].rearrange("p (h c) -> p h c", c=65)[:, hg]),
                                start=False, stop=False)
                            nc.tensor.matmul(
                                psv, mm(cur_e[:, 0:W]),
                                mm(v_ext[w + 1].rearrange("p (h c) -> p h c", c=65)[:, hg]),
                                start=False, stop=True)
                            rc = rc_p.tile([128, 1], f32, tag="rc", name="rc")
                            nc.vector.reciprocal(rc, psv[:, 64:65])
                            nc.vector.tensor_scalar_mul(
                                out_all[w][:, hg * 64:(hg + 1) * 64],
                                psv[:, 0:64], rc)

            # ---- phase 4: out transpose + final projection ----
            for w in range(NWIN):
                outT = [ot_p.tile([128, 128], f32, tag="ot", name=f"ot{_}") for _ in range(DC8)]
                for d8 in range(DC8):
                    ps = tr_p.tile([128, 128], f32, tag="tr", name="tr")
                    nc.tensor.transpose(
                        ps, out_all[w][:, d8 * 128:(d8 + 1) * 128], ident)
                    nc.vector.tensor_copy(outT[d8], ps)
                ysb = y_p.tile([128, DIM], f32, tag="y", name="y")
                for nn in range(2):
                    ps = pp.tile([128, 512], f32, tag="pp", name="pp")[:, :384]
                    for d8 in range(DC8):
                        nc.tensor.matmul(
                            ps, mm(outT[d8]), mm(wo_sb[d8][:, nn * 384:(nn + 1) * 384]),
                            start=(d8 == 0), stop=(d8 == DC8 - 1))
                    nc.vector.tensor_copy(ysb[:, nn * 384:(nn + 1) * 384], ps)
                nc.sync.dma_start(
                    y_d[b * TOK + w * W:b * TOK + (w + 1) * W, :], ysb)
    nc.compile()
    return nc


def host_prep(x, mask, attn_bias, Wq, bq, Wkv, Wo, memory_kv):
    s = np.float32(DH ** -0.5)
    wq = (np.asarray(Wq, np.float32) * s).astype(np.float32)
    bqs = (np.asarray(bq, np.float32) * s).astype(np.float32).reshape(DC8, 128)
    wkv = np.ascontiguousarray(np.asarray(Wkv, np.float32))
    wo = np.ascontiguousarray(np.asarray(Wo, np.float32))
    x = np.asarray(x, np.float32)
    mask = np.asarray(mask).astype(bool)
    attn_bias = np.asarray(attn_bias, np.float32)
    mk = np.asarray(memory_kv[0], np.float32)
    mv = np.asarray(memory_kv[1], np.float32)

    memk = np.zeros((128, 32), np.float32)
    for hp in range(8):
        memk[0:64, hp * 4:(hp + 1) * 4] = mk[2 * hp].T
        memk[64:128, hp * 4:(hp + 1) * 4] = mk[2 * hp + 1].T
    memv = np.zeros((4, 16 * 65), np.float32)
    for h in range(H):
        memv[:, h * 65:h * 65 + 64] = mv[h]
        memv[:, h * 65 + 64] = 1.0

    shared = dict(wq=wq, bqs=bqs, wkv=wkv, wo=wo, memk=memk, memv=memv)
    in_maps = []
    for c in range(NCORES):
        q0 = c * TOK
        xkv = np.zeros((B, KTOK, DIM), np.float32)
        lo = q0 - W
        src_lo = max(lo, 0)
        xkv[:, src_lo - lo:, :] = x[:, src_lo:q0 + TOK, :]
        biasc = np.full((B, NKC, W, 2 * W), NEG, np.float32)
        for b in range(B):
            for kc in range(NKC):
                gk = c * NWIN + kc - 1
                if gk < 0:
                    continue
                kr = slice(gk * W, (gk + 1) * W)
                if kc >= 1:
                    qr = slice((c * NWIN + kc - 1) * W, (c * NWIN + kc) * W)
                    biasc[b, kc, :, 0:W] = attn_bias[b, qr, kr].T
                if kc <= NWIN - 1:
                    qr = slice((c * NWIN + kc) * W, (c * NWIN + kc + 1) * W)
                    biasc[b, kc, :, W:2 * W] = attn_bias[b, qr, kr].T
                kmask = mask[b, gk * W:(gk + 1) * W]
                biasc[b, kc, ~kmask, :] = NEG
        in_maps.append(dict(
            xkv=np.ascontiguousarray(xkv.reshape(B * KTOK, DIM)),
            biasc=np.ascontiguousarray(biasc.reshape(B * NKC * W, 2 * W)),
            **shared))
    return in_maps


_CACHE = {}


def kernel(**inputs):
    import sys
    if "/opt/trn_rl_repo" not in sys.path:
        sys.path.insert(0, "/opt/trn_rl_repo")
    from concourse.bass_utils import run_bass_kernel_spmd

    in_maps = host_prep(**inputs)
    if "nc" not in _CACHE:
        _CACHE["nc"] = build_bass()
    nc = _CACHE["nc"]
    res = run_bass_kernel_spmd(nc, in_maps, core_ids=list(range(NCORES)))
    ys = [res.results[c]["y"].reshape(B, TOK, DIM) for c in range(NCORES)]
    return np.concatenate(ys, axis=1)


if __name__ == "__main__":
    import sys
    sys.path.insert(0, "/opt/trn_rl_repo")
    nc = build_bass()
    print("build OK")
